# revision 9
# baseline (speedup 1.0000x reference)
"""Trainium2 Bass kernel for SAM-style decomposed rel-pos attention (v3).

Problem: B=1, HW=2304 (48x48), NH=16 heads, DH=64, D=1024, f32 in/out.
  attn = softmax(q*scale @ k^T + rel_h[qh,kh] + rel_w[qw,kw]); out = attn @ v

Strategy (8 NeuronCores, SPMD): 2 heads per core. Key ideas:
- rel_h is folded into the single score matmul per k-tile (one-hot Eh rows
  + K^T stacked as the stationary operand; gathered rel_h^T rows + Q^T as
  the moving operand) -> 18 score matmuls per (head, chunk) instead of 36.
- rel_w is applied MULTIPLICATIVELY after exp: P = exp(S_qk+relh) * Ew
  where Ew[k,q] = exp(rel_w^T[kw(k), q]). kw(k) is periodic with period 48
  and 128 = 2*48 + 32, so only 3 row-rotations (offsets 0/32/16 = kt mod 3)
  of exp_relw exist -> a [128, 3, HW] "patterns" tile serves every k-tile
  triple via one DVE tensor_mul per 3-k-tile group.
- Diagonal gathers (rel tables are banded) are ONE DMA each via a DRAM
  roundtrip: T1 tables stored to scratch DRAM, re-loaded with a 3D access
  pattern whose middle dim strides -2256 (one row up, 48 cols right).
- exp on ScalarE in [128, 3, qn] groups from PSUM; PV matmuls run PV_LAG
  groups behind the score matmuls; the reciprocal-broadcast matmul of each
  chunk's epilogue is deferred further so the in-order PE stream never
  waits -> PE_HAM un-throttles the clock 1.2 -> 2.4 GHz.
- Head 1's prep (table matmuls, stores, gathers, exp, patterns) is
  interleaved into head 0's main loop at chunk boundaries so the PE and
  DMA rings stay busy; head 0's prep is the only exposed startup.
- Softmax denominator: ones-column in V_aug -> row 64 of the PV output;
  reciprocal on a [128, 4] transposed view (two tiny DMAs) since a
  [1, 512] single-partition reciprocal wastes 127 DVE lanes.
"""

import sys

sys.path.insert(0, "/opt/trn_rl_repo")

import numpy as np
import ml_dtypes

from concourse import bacc, mybir, tile
from concourse.tile import add_dep_helper
from concourse.bass_utils import run_bass_kernel_spmd

BF16 = mybir.dt.bfloat16
F32 = mybir.dt.float32
BF = ml_dtypes.bfloat16

H = 48
W = 48
HW = H * W          # 2304
DH = 64
NH = 16
N_CORES = 8
HPC = 2             # heads per core
KT = HW // 128      # 18 k tiles
QCHUNKS = [(0, 512), (512, 512), (1024, 512), (1536, 512), (2048, 256)]
NG = KT // 3        # 6 groups of 3 k-tiles
PV_LAG = 3          # PV runs this many groups behind the score matmuls
RB_LAG = 5          # reciprocal-broadcast matmul deferral (groups)

_NC = None


def _build_nc():
    nc = bacc.Bacc(None, target_bir_lowering=False)

    q_t = nc.dram_tensor("q_t", [128, HW], BF16, kind="ExternalInput")
    qw_t = nc.dram_tensor("qw_t", [128, HW], BF16, kind="ExternalInput")
    k_t = nc.dram_tensor("k_t", [128, HW], BF16, kind="ExternalInput")
    v_til = nc.dram_tensor("v_til", [128, HPC * KT * 65], BF16, kind="ExternalInput")
    rhv = nc.dram_tensor("rhv", [64, 95], BF16, kind="ExternalInput")
    rwv = nc.dram_tensor("rwv", [64, 95], BF16, kind="ExternalInput")
    eh = nc.dram_tensor("eh", [64, HW], BF16, kind="ExternalInput")
    out_t = nc.dram_tensor("out_t", [128, HW], F32, kind="ExternalOutput")
    t1d = [nc.dram_tensor(f"t1d{h}", [95, HW], BF16, kind="Internal") for h in range(HPC)]
    t2d = [nc.dram_tensor(f"t2d{h}", [95, HW], BF16, kind="Internal") for h in range(HPC)]

    Exp = mybir.ActivationFunctionType.Exp

    with tile.TileContext(nc) as tc:
        with (
            tc.tile_pool(name="const", bufs=1) as cpool,
            tc.tile_pool(name="stack", bufs=2) as spool,
            tc.tile_pool(name="p1t", bufs=3) as p1pool,
            tc.tile_pool(name="p2t", bufs=4) as p2pool,
            tc.tile_pool(name="epil", bufs=3) as epool,
            tc.tile_pool(name="ps_s", bufs=2, space="PSUM") as ps_s,
            tc.tile_pool(name="ps_o", bufs=2, space="PSUM") as ps_o,
        ):
            rhv_sb = cpool.tile([128, 95], BF16, tag="rhv")
            rwv_sb = cpool.tile([128, 95], BF16, tag="rwv")
            ones1 = cpool.tile([1, 64], BF16, tag="ones1")
            nc.sync.dma_start(rhv_sb[64:128, :], rhv[:, :])
            nc.sync.dma_start(rwv_sb[64:128, :], rwv[:, :])
            nc.gpsimd.memset(ones1[:], 1.0)

            # per-head state dicts
            hs = [dict() for _ in range(HPC)]

            def ph1_load(hh):
                dmae = nc.sync if hh == 0 else nc.gpsimd
                c0, c1 = hh * 64, (hh + 1) * 64
                s = hs[hh]
                s["c0"], s["c1"] = c0, c1
                s["lhsT"] = spool.tile([128, HW], BF16, tag="lhsT", name=f"lhsT{hh}")
                s["rhs"] = spool.tile([128, HW], BF16, tag="rhs", name=f"rhs{hh}")
                s["qwt"] = spool.tile([128, HW], BF16, tag="qwt", name=f"qwt{hh}")
                s["vt"] = spool.tile([128, KT * 65], BF16, tag="vt", name=f"vt{hh}")
                s["t1h"] = spool.tile([95, HW], BF16, tag="t1h", name=f"t1h{hh}")
                s["t2w"] = spool.tile([95, HW], BF16, tag="t2w", name=f"t2w{hh}")
                s["relw"] = spool.tile([48, HW], BF16, tag="relw", name=f"relw{hh}")
                s["relq"] = spool.tile([48, HW], BF16, tag="relq", name=f"relq{hh}")
                s["expw"] = spool.tile([48, HW], BF16, tag="expw", name=f"expw{hh}")
                s["pats"] = spool.tile([128, 3, HW], BF16, tag="pats", name=f"pats{hh}")
                dmae.dma_start(s["lhsT"][0:64, :], eh[:, :])
                dmae.dma_start(s["lhsT"][64:128, :], k_t[c0:c1, :])
                dmae.dma_start(s["rhs"][48:64, :], eh[48:64, :])   # zeros
                dmae.dma_start(s["rhs"][64:128, :], q_t[c0:c1, :])
                dmae.dma_start(s["qwt"][64:128, :], qw_t[c0:c1, :])
                dmae.dma_start(s["vt"][:, :], v_til[:, hh * KT * 65 : (hh + 1) * KT * 65])

            def ph1_t2w(hh, lo, hi):
                # T2w[r, q'] = sum_c 8*rpw[94-r, c] * qw[c, q']   (w-major cols)
                s = hs[hh]
                cpe = nc.vector
                for (q0, qn) in QCHUNKS[lo:hi]:
                    tp = ps_s.tile([128, 3, 512], F32, tag="s", name=f"tpw{hh}_{q0}")
                    nc.tensor.matmul(
                        tp[0:95, 0, 0:qn], rwv_sb[64:128, :],
                        s["qwt"][64:128, q0 : q0 + qn], start=True, stop=True,
                    )
                    cpe.tensor_copy(s["t2w"][:, q0 : q0 + qn], tp[0:95, 0, 0:qn])

            def ph1_t1h(hh, lo, hi):
                s = hs[hh]
                cpe = nc.vector
                for (q0, qn) in QCHUNKS[lo:hi]:
                    tp = ps_s.tile([128, 3, 512], F32, tag="s", name=f"tph{hh}_{q0}")
                    nc.tensor.matmul(
                        tp[0:95, 0, 0:qn], rhv_sb[64:128, :],
                        s["rhs"][64:128, q0 : q0 + qn], start=True, stop=True,
                    )
                    cpe.tensor_copy(s["t1h"][:, q0 : q0 + qn], tp[0:95, 0, 0:qn])

            def ph1_t2_store_gather(hh):
                s = hs[hh]
                # store halves on both rings, then one diagonal gather
                nc.sync.dma_start(t2d[hh][0:48, :], s["t2w"][0:48, :])
                nc.gpsimd.dma_start(t2d[hh][48:95, :], s["t2w"][48:95, :])
                dstw = s["relw"][0:48, :].rearrange("p (w h) -> p w h", h=48)
                srcw = t2d[hh][47:95, 0:HW].rearrange("j (w h) -> j w h", h=48)
                srcw.ap[1] = [-2256, 48]
                (nc.sync if hh == 0 else nc.gpsimd).dma_start(dstw, srcw)

            def ph1_t1_store_gather(hh):
                s = hs[hh]
                nc.sync.dma_start(t1d[hh][0:48, :], s["t1h"][0:48, :])
                nc.gpsimd.dma_start(t1d[hh][48:95, :], s["t1h"][48:95, :])
                dsth = s["rhs"][0:48, :].rearrange("p (h w) -> p h w", w=48)
                srch = t1d[hh][47:95, 0:HW].rearrange("j (h w) -> j h w", w=48)
                srch.ap[1] = [-2256, 48]
                (nc.sync if hh == 0 else nc.gpsimd).dma_start(dsth, srch)

            def ph1_relw_tail(hh, half):
                # un-permute w-major relw -> q-major (col half), exp, patterns
                s = hs[hh]
                h0 = half * 24  # h-blocks 0:24 / 24:48  -> cols 0:1152 / 1152:2304
                co, cn = h0 * 48, 1152
                nc.vector.tensor_copy(
                    s["relq"][:, co : co + cn].rearrange("p (h w) -> p h w", w=48),
                    s["relw"][:, :].rearrange("p (w h) -> p h w", w=48)[:, h0 : h0 + 24, :],
                )
                nc.scalar.activation(
                    s["expw"][:, co : co + cn], s["relq"][:, co : co + cn], Exp
                )
                ndma = 0
                for j, off in enumerate((0, 32, 16)):
                    p = 0
                    while p < 128:
                        r0 = (p + off) % 48
                        n = min(48 - r0, 128 - p)
                        eng = (nc.sync, nc.gpsimd)[(hh + ndma) % 2]
                        eng.dma_start(
                            s["pats"][p : p + n, j, co : co + cn],
                            s["expw"][r0 : r0 + n, co : co + cn],
                        )
                        p += n
                        ndma += 1

            # ---- main loop machinery (per head) ----
            def make_main(hh):
                s = hs[hh]
                st = {"pend": [], "epiB": [], "g": 0}

                def epilogue_a(ci, q0, qn, o_ps):
                    den = epool.tile([1, 512], F32, tag="den", name=f"den{hh}_{ci}")
                    nc.vector.tensor_copy(den[0:1, 0:qn], o_ps[64:65, 0:qn])
                    dmae = nc.sync if ci % 2 == 0 else nc.gpsimd
                    dent = epool.tile([128, 4], F32, tag="dent", name=f"dent{hh}_{ci}")
                    nr = qn // 128
                    dmae.dma_start(dent[0:128, 0:nr], den[0:1, 0:qn])
                    rect = epool.tile([128, 4], F32, tag="rect", name=f"rect{hh}_{ci}")
                    nc.vector.reciprocal(rect[0:128, 0:nr], dent[0:128, 0:nr])
                    recb = epool.tile([128, 4], BF16, tag="recb", name=f"recb{hh}_{ci}")
                    nc.vector.tensor_copy(recb[0:128, 0:nr], rect[0:128, 0:nr])
                    recr = epool.tile([1, 512], BF16, tag="recr", name=f"recr{hh}_{ci}")
                    dmae.dma_start(recr[0:1, 0:qn], recb[0:128, 0:nr])
                    return recr

                def epilogue_b(ci, q0, qn, o_ps, recr):
                    rb_ps = ps_s.tile([128, 3, 512], F32, tag="s", name=f"rb{hh}_{ci}")
                    nc.tensor.matmul(
                        rb_ps[0:64, 0, 0:qn], ones1[:], recr[0:1, 0:qn],
                        start=True, stop=True,
                    )
                    rb_sb = epool.tile([64, 512], F32, tag="rb_sb", name=f"rbs{hh}_{ci}")
                    nc.vector.tensor_copy(rb_sb[:, 0:qn], rb_ps[0:64, 0, 0:qn])
                    ot = epool.tile([64, 512], F32, tag="ot", name=f"ot{hh}_{ci}")
                    nc.vector.tensor_mul(
                        ot[:, 0:qn], o_ps[0:64, 0:qn], rb_sb[:, 0:qn]
                    )
                    nc.sync.dma_start(out_t[s["c0"] : s["c1"], q0 : q0 + qn], ot[:, 0:qn])

                def flush(last_mm, keep):
                    while len(st["pend"]) > keep:
                        (ci, q0, qn, o_ps, g, p2) = st["pend"].pop(0)
                        for j in range(3):
                            kt = 3 * g + j
                            pv = nc.tensor.matmul(
                                o_ps[0:65, 0:qn],
                                s["vt"][:, kt * 65 : (kt + 1) * 65],
                                p2[:, j, 0:qn],
                                start=(kt == 0), stop=(kt == KT - 1),
                            )
                            if last_mm is not None:
                                add_dep_helper(pv.ins, last_mm.ins, sync=False,
                                               reason="pv after score mms")
                        if g == NG - 1:
                            recr = epilogue_a(ci, q0, qn, o_ps)
                            st["epiB"].append((ci, q0, qn, o_ps, recr, st["g"] + RB_LAG))
                    while st["epiB"] and st["epiB"][0][5] <= st["g"]:
                        (ci, q0, qn, o_ps, recr, _) = st["epiB"].pop(0)
                        epilogue_b(ci, q0, qn, o_ps, recr)

                def chunk(ci):
                    (q0, qn) = QCHUNKS[ci]
                    o_ps = ps_o.tile([65, 512], F32, tag="o", name=f"o{hh}_{ci}")
                    for g in range(NG):
                        s_ps = ps_s.tile([128, 3, 512], F32, tag="s",
                                         name=f"s{hh}_{ci}_{g}")
                        last_mm = None
                        for j in range(3):
                            kt = 3 * g + j
                            last_mm = nc.tensor.matmul(
                                s_ps[:, j, 0:qn],
                                s["lhsT"][:, kt * 128 : (kt + 1) * 128],
                                s["rhs"][:, q0 : q0 + qn],
                                start=True, stop=True,
                            )
                        st["g"] += 1
                        flush(last_mm, PV_LAG - 1)
                        p1 = p1pool.tile([128, 3, 512], BF16, tag="p1")
                        nc.scalar.activation(p1[:, :, 0:qn], s_ps[:, :, 0:qn], Exp)
                        p2 = p2pool.tile([128, 3, 512], BF16, tag="p2")
                        nc.vector.tensor_mul(
                            p2[:, :, 0:qn], p1[:, :, 0:qn],
                            s["pats"][:, :, q0 : q0 + qn],
                        )
                        st["pend"].append((ci, q0, qn, o_ps, g, p2))

                def finish():
                    st["g"] += PV_LAG
                    flush(None, 0)
                    st["g"] += RB_LAG + NG
                    flush(None, 0)

                return chunk, finish

            # ---- schedule ----
            ph1_load(0)
            ph1_load(1)
            ph1_t2w(0, 0, 5)
            ph1_t2_store_gather(0)
            ph1_t1h(0, 0, 5)
            ph1_t1_store_gather(0)
            ph1_relw_tail(0, 0)
            ph1_relw_tail(0, 1)
            chunk0, finish0 = make_main(0)
            chunk1, finish1 = make_main(1)
            chunk0(0)
            ph1_t2w(1, 0, 3)
            chunk0(1)
            ph1_t2w(1, 3, 5)
            ph1_t2_store_gather(1)
            chunk0(2)
            ph1_t1h(1, 0, 3)
            ph1_relw_tail(1, 0)
            chunk0(3)
            ph1_t1h(1, 3, 5)
            ph1_t1_store_gather(1)
            ph1_relw_tail(1, 1)
            chunk0(4)
            finish0()
            for ci in range(5):
                chunk1(ci)
            finish1()

    nc.compile()
    return nc


def _get_nc():
    global _NC
    if _NC is None:
        _NC = _build_nc()
    return _NC


def _host_prep(q, k, v, rel_pos_h, rel_pos_w):
    q2 = np.asarray(q, np.float32).reshape(HW, NH * DH)
    k2 = np.asarray(k, np.float32).reshape(HW, NH * DH)
    v2 = np.asarray(v, np.float32).reshape(HW, NH * DH)
    rph = np.asarray(rel_pos_h, np.float32)
    rpw = np.asarray(rel_pos_w, np.float32)

    ar = np.arange(48)
    # reversed rel tables, x8 cancels the 0.125 q scale
    rhv = np.ascontiguousarray((8.0 * rph[::-1]).T).astype(BF)   # (64, 95)
    rwv = np.ascontiguousarray((8.0 * rpw[::-1]).T).astype(BF)
    kk = np.arange(HW)
    eh = np.zeros((64, HW), np.float32)
    eh[:48] = kk[None, :] // 48 == ar[:, None]
    eh = eh.astype(BF)

    onecol = np.ones((HW, 1), np.float32)
    in_maps = []
    for c in range(N_CORES):
        sl = slice(c * 128, (c + 1) * 128)
        qs = (q2[:, sl].T * 0.125).astype(BF)                    # (128, HW)
        qw = np.ascontiguousarray(
            qs.reshape(128, 48, 48).transpose(0, 2, 1)
        ).reshape(128, HW)                                       # w-major cols
        ks = k2[:, sl].T.astype(BF)
        vparts = []
        for hh in range(HPC):
            vh = v2[:, c * 128 + hh * 64 : c * 128 + (hh + 1) * 64]
            va = np.concatenate([vh, onecol], axis=1)            # (HW, 65)
            vparts.append(va.reshape(KT, 128, 65).transpose(1, 0, 2).reshape(128, KT * 65))
        v_til = np.concatenate(vparts, axis=1).astype(BF)        # (128, 2*18*65)
        in_maps.append(
            dict(q_t=qs, qw_t=qw, k_t=ks, v_til=v_til, rhv=rhv, rwv=rwv, eh=eh)
        )
    return in_maps


def _assemble(results):
    cols = [np.asarray(r["out_t"], np.float32).T for r in results]  # (HW, 128) each
    return np.concatenate(cols, axis=1).reshape(1, H, W, NH * DH)


def kernel(q, k, v, rel_pos_h, rel_pos_w):
    nc = _get_nc()
    in_maps = _host_prep(q, k, v, rel_pos_h, rel_pos_w)
    res = run_bass_kernel_spmd(nc, in_maps, core_ids=list(range(N_CORES)))
    return _assemble(res.results)


# revision 12
# speedup vs baseline: 1.0103x; 1.0103x over previous
"""Trainium2 Bass kernel for SAM-style decomposed rel-pos attention (v3).

Problem: B=1, HW=2304 (48x48), NH=16 heads, DH=64, D=1024, f32 in/out.
  attn = softmax(q*scale @ k^T + rel_h[qh,kh] + rel_w[qw,kw]); out = attn @ v

Strategy (8 NeuronCores, SPMD): 2 heads per core. Key ideas:
- rel_h is folded into the single score matmul per k-tile (one-hot Eh rows
  + K^T stacked as the stationary operand; gathered rel_h^T rows + Q^T as
  the moving operand) -> 18 score matmuls per (head, chunk) instead of 36.
- rel_w is applied MULTIPLICATIVELY after exp: P = exp(S_qk+relh) * Ew
  where Ew[k,q] = exp(rel_w^T[kw(k), q]). kw(k) is periodic with period 48
  and 128 = 2*48 + 32, so only 3 row-rotations (offsets 0/32/16 = kt mod 3)
  of exp_relw exist -> a [128, 3, HW] "patterns" tile serves every k-tile
  triple via one DVE tensor_mul per 3-k-tile group.
- Diagonal gathers (rel tables are banded) are ONE DMA each via a DRAM
  roundtrip: T1 tables stored to scratch DRAM, re-loaded with a 3D access
  pattern whose middle dim strides -2256 (one row up, 48 cols right).
- exp on ScalarE in [128, 3, qn] groups from PSUM; PV matmuls run PV_LAG
  groups behind the score matmuls; the reciprocal-broadcast matmul of each
  chunk's epilogue is deferred further so the in-order PE stream never
  waits -> PE_HAM un-throttles the clock 1.2 -> 2.4 GHz.
- Head 1's prep (table matmuls, stores, gathers, exp, patterns) is
  interleaved into head 0's main loop at chunk boundaries so the PE and
  DMA rings stay busy; head 0's prep is the only exposed startup.
- Softmax denominator: ones-column in V_aug -> row 64 of the PV output;
  reciprocal on a [128, 4] transposed view (two tiny DMAs) since a
  [1, 512] single-partition reciprocal wastes 127 DVE lanes.
"""

import sys

sys.path.insert(0, "/opt/trn_rl_repo")

import numpy as np
import ml_dtypes

from concourse import bacc, mybir, tile
from concourse.tile import add_dep_helper
from concourse.bass_utils import run_bass_kernel_spmd

BF16 = mybir.dt.bfloat16
F32 = mybir.dt.float32
BF = ml_dtypes.bfloat16

H = 48
W = 48
HW = H * W          # 2304
DH = 64
NH = 16
N_CORES = 8
HPC = 2             # heads per core
KT = HW // 128      # 18 k tiles
QCHUNKS = [(0, 512), (512, 512), (1024, 512), (1536, 512), (2048, 256)]
NG = KT // 3        # 6 groups of 3 k-tiles
PV_LAG = 3          # PV runs this many groups behind the score matmuls
RB_LAG = 5          # reciprocal-broadcast matmul deferral (groups)

_NC = None


def _build_nc():
    nc = bacc.Bacc(None, target_bir_lowering=False)

    q_t = nc.dram_tensor("q_t", [128, HW], BF16, kind="ExternalInput")
    qw_t = nc.dram_tensor("qw_t", [128, HW], BF16, kind="ExternalInput")
    k_t = nc.dram_tensor("k_t", [128, HW], BF16, kind="ExternalInput")
    v_til = nc.dram_tensor("v_til", [128, HPC * KT * 65], BF16, kind="ExternalInput")
    rhv = nc.dram_tensor("rhv", [64, 95], BF16, kind="ExternalInput")
    rwv = nc.dram_tensor("rwv", [64, 95], BF16, kind="ExternalInput")
    eh = nc.dram_tensor("eh", [64, HW], BF16, kind="ExternalInput")
    out_t = nc.dram_tensor("out_t", [128, HW], F32, kind="ExternalOutput")
    t1d = [nc.dram_tensor(f"t1d{h}", [95, HW], BF16, kind="Internal") for h in range(HPC)]
    t2d = [nc.dram_tensor(f"t2d{h}", [95, HW], BF16, kind="Internal") for h in range(HPC)]

    Exp = mybir.ActivationFunctionType.Exp

    with tile.TileContext(nc) as tc:
        with (
            tc.tile_pool(name="const", bufs=1) as cpool,
            tc.tile_pool(name="stack", bufs=2) as spool,
            tc.tile_pool(name="p1t", bufs=3) as p1pool,
            tc.tile_pool(name="p2t", bufs=4) as p2pool,
            tc.tile_pool(name="epil", bufs=3) as epool,
            tc.tile_pool(name="ps_s", bufs=2, space="PSUM") as ps_s,
            tc.tile_pool(name="ps_o", bufs=2, space="PSUM") as ps_o,
        ):
            rhv_sb = cpool.tile([128, 95], BF16, tag="rhv")
            rwv_sb = cpool.tile([128, 95], BF16, tag="rwv")
            ones1 = cpool.tile([1, 64], BF16, tag="ones1")
            nc.sync.dma_start(rhv_sb[64:128, :], rhv[:, :])
            nc.sync.dma_start(rwv_sb[64:128, :], rwv[:, :])
            nc.gpsimd.memset(ones1[:], 1.0)

            # per-head state dicts
            hs = [dict() for _ in range(HPC)]

            def ph1_load(hh):
                dmae = nc.sync if hh == 0 else nc.gpsimd
                c0, c1 = hh * 64, (hh + 1) * 64
                s = hs[hh]
                s["c0"], s["c1"] = c0, c1
                s["lhsT"] = spool.tile([128, HW], BF16, tag="lhsT", name=f"lhsT{hh}")
                s["rhs"] = spool.tile([128, HW], BF16, tag="rhs", name=f"rhs{hh}")
                s["qwt"] = spool.tile([128, HW], BF16, tag="qwt", name=f"qwt{hh}")
                s["vt"] = spool.tile([128, KT * 65], BF16, tag="vt", name=f"vt{hh}")
                s["t12"] = spool.tile([95, 2, HW], BF16, tag="t12", name=f"t12{hh}")
                s["relw"] = spool.tile([48, HW], BF16, tag="relw", name=f"relw{hh}")
                s["relq"] = spool.tile([48, HW], BF16, tag="relq", name=f"relq{hh}")
                s["expw"] = spool.tile([48, HW], BF16, tag="expw", name=f"expw{hh}")
                s["pats"] = spool.tile([128, 3, HW], BF16, tag="pats", name=f"pats{hh}")
                dmae.dma_start(s["lhsT"][0:64, :], eh[:, :])
                dmae.dma_start(s["lhsT"][64:128, :], k_t[c0:c1, :])
                dmae.dma_start(s["rhs"][48:64, :], eh[48:64, :])   # zeros
                dmae.dma_start(s["rhs"][64:128, :], q_t[c0:c1, :])
                dmae.dma_start(s["qwt"][64:128, :], qw_t[c0:c1, :])
                dmae.dma_start(s["vt"][:, :], v_til[:, hh * KT * 65 : (hh + 1) * KT * 65])

            # gather piece ranges: piece c covers blocks [lo, hi) whose cols
            # fit inside table-store chunks 0..c (512 cols each)
            PIECES = [(0, 10), (10, 21), (21, 32), (32, 42), (42, 48)]

            def ph1_tabs(hh, lo, hi):
                # per chunk: T2w (w-major rel_w table) and T1h (rel_h table)
                # matmuls -> one fused cast -> chunked DRAM stores -> gather
                # piece. t12[:, 0, :] = T2w, t12[:, 1, :] = T1h.
                s = hs[hh]
                for ci in range(lo, hi):
                    (q0, qn) = QCHUNKS[ci]
                    tp = ps_s.tile([128, 3, 512], F32, tag="s", name=f"tp{hh}_{ci}")
                    nc.tensor.matmul(
                        tp[0:95, 0, 0:qn], rwv_sb[64:128, :],
                        s["qwt"][64:128, q0 : q0 + qn], start=True, stop=True,
                    )
                    nc.tensor.matmul(
                        tp[0:95, 1, 0:qn], rhv_sb[64:128, :],
                        s["rhs"][64:128, q0 : q0 + qn], start=True, stop=True,
                    )
                    nc.vector.tensor_copy(
                        s["t12"][:, :, q0 : q0 + qn], tp[0:95, 0:2, 0:qn]
                    )
                    # chunked stores, partition-split across rings
                    for tbl, dram in ((0, t2d[hh]), (1, t1d[hh])):
                        nc.sync.dma_start(
                            dram[0:48, q0 : q0 + qn], s["t12"][0:48, tbl, q0 : q0 + qn]
                        )
                        nc.gpsimd.dma_start(
                            dram[48:95, q0 : q0 + qn], s["t12"][48:95, tbl, q0 : q0 + qn]
                        )
                    # gather pieces (diagonal re-load)
                    (ba, bb) = PIECES[ci]
                    nbl = bb - ba
                    # rel_h: rhs[j, (h,w)] = T1h[47-h+j, 48h+w], h in [ba, bb)
                    dsth = s["rhs"][0:48, 48 * ba : 48 * bb].rearrange(
                        "p (h w) -> p h w", w=48
                    )
                    srch = t1d[hh][47 - ba : 95 - ba, 48 * ba : 48 * bb].rearrange(
                        "j (h w) -> j h w", w=48
                    )
                    srch.ap[1] = [-2256, nbl]
                    nc.sync.dma_start(dsth, srch)
                    # rel_w (w-major): relw[j, (w,h)] = T2w[47-w+j, 48w+h]
                    dstw = s["relw"][0:48, 48 * ba : 48 * bb].rearrange(
                        "p (w h) -> p w h", h=48
                    )
                    srcw = t2d[hh][47 - ba : 95 - ba, 48 * ba : 48 * bb].rearrange(
                        "j (w h) -> j w h", h=48
                    )
                    srcw.ap[1] = [-2256, nbl]
                    nc.gpsimd.dma_start(dstw, srcw)

            def ph1_relw_tail(hh, half):
                # un-permute w-major relw -> q-major (col half), exp, patterns
                s = hs[hh]
                h0 = half * 24  # h-blocks 0:24 / 24:48  -> cols 0:1152 / 1152:2304
                co, cn = h0 * 48, 1152
                nc.vector.tensor_copy(
                    s["relq"][:, co : co + cn].rearrange("p (h w) -> p h w", w=48),
                    s["relw"][:, :].rearrange("p (w h) -> p h w", w=48)[:, h0 : h0 + 24, :],
                )
                nc.scalar.activation(
                    s["expw"][:, co : co + cn], s["relq"][:, co : co + cn], Exp
                )
                ndma = 0
                for j, off in enumerate((0, 32, 16)):
                    p = 0
                    while p < 128:
                        r0 = (p + off) % 48
                        n = min(48 - r0, 128 - p)
                        eng = (nc.sync, nc.gpsimd)[(hh + ndma) % 2]
                        eng.dma_start(
                            s["pats"][p : p + n, j, co : co + cn],
                            s["expw"][r0 : r0 + n, co : co + cn],
                        )
                        p += n
                        ndma += 1

            # ---- main loop machinery (per head) ----
            def make_main(hh):
                s = hs[hh]
                st = {"pend": [], "epiB": [], "g": 0}

                def epilogue_a(ci, q0, qn, o_ps):
                    den = epool.tile([1, 512], F32, tag="den", name=f"den{hh}_{ci}")
                    nc.vector.tensor_copy(den[0:1, 0:qn], o_ps[64:65, 0:qn])
                    dmae = nc.sync if ci % 2 == 0 else nc.gpsimd
                    dent = epool.tile([128, 4], F32, tag="dent", name=f"dent{hh}_{ci}")
                    nr = qn // 128
                    dmae.dma_start(dent[0:128, 0:nr], den[0:1, 0:qn])
                    rect = epool.tile([128, 4], F32, tag="rect", name=f"rect{hh}_{ci}")
                    nc.vector.reciprocal(rect[0:128, 0:nr], dent[0:128, 0:nr])
                    recb = epool.tile([128, 4], BF16, tag="recb", name=f"recb{hh}_{ci}")
                    nc.vector.tensor_copy(recb[0:128, 0:nr], rect[0:128, 0:nr])
                    recr = epool.tile([1, 512], BF16, tag="recr", name=f"recr{hh}_{ci}")
                    dmae.dma_start(recr[0:1, 0:qn], recb[0:128, 0:nr])
                    return recr

                def epilogue_b(ci, q0, qn, o_ps, recr):
                    rb_ps = ps_s.tile([128, 3, 512], F32, tag="s", name=f"rb{hh}_{ci}")
                    nc.tensor.matmul(
                        rb_ps[0:64, 0, 0:qn], ones1[:], recr[0:1, 0:qn],
                        start=True, stop=True,
                    )
                    rb_sb = epool.tile([64, 512], F32, tag="rb_sb", name=f"rbs{hh}_{ci}")
                    nc.vector.tensor_copy(rb_sb[:, 0:qn], rb_ps[0:64, 0, 0:qn])
                    ot = epool.tile([64, 512], F32, tag="ot", name=f"ot{hh}_{ci}")
                    nc.vector.tensor_mul(
                        ot[:, 0:qn], o_ps[0:64, 0:qn], rb_sb[:, 0:qn]
                    )
                    nc.sync.dma_start(out_t[s["c0"] : s["c1"], q0 : q0 + qn], ot[:, 0:qn])

                def flush(last_mm, keep):
                    while len(st["pend"]) > keep:
                        (ci, q0, qn, o_ps, g, p2) = st["pend"].pop(0)
                        for j in range(3):
                            kt = 3 * g + j
                            pv = nc.tensor.matmul(
                                o_ps[0:65, 0:qn],
                                s["vt"][:, kt * 65 : (kt + 1) * 65],
                                p2[:, j, 0:qn],
                                start=(kt == 0), stop=(kt == KT - 1),
                            )
                            if last_mm is not None:
                                add_dep_helper(pv.ins, last_mm.ins, sync=False,
                                               reason="pv after score mms")
                        if g == NG - 1:
                            recr = epilogue_a(ci, q0, qn, o_ps)
                            st["epiB"].append((ci, q0, qn, o_ps, recr, st["g"] + RB_LAG))
                    while st["epiB"] and st["epiB"][0][5] <= st["g"]:
                        (ci, q0, qn, o_ps, recr, _) = st["epiB"].pop(0)
                        epilogue_b(ci, q0, qn, o_ps, recr)

                def chunk(ci):
                    (q0, qn) = QCHUNKS[ci]
                    o_ps = ps_o.tile([65, 512], F32, tag="o", name=f"o{hh}_{ci}")
                    for g in range(NG):
                        s_ps = ps_s.tile([128, 3, 512], F32, tag="s",
                                         name=f"s{hh}_{ci}_{g}")
                        last_mm = None
                        for j in range(3):
                            kt = 3 * g + j
                            last_mm = nc.tensor.matmul(
                                s_ps[:, j, 0:qn],
                                s["lhsT"][:, kt * 128 : (kt + 1) * 128],
                                s["rhs"][:, q0 : q0 + qn],
                                start=True, stop=True,
                            )
                        st["g"] += 1
                        flush(last_mm, PV_LAG - 1)
                        p1 = p1pool.tile([128, 3, 512], BF16, tag="p1")
                        nc.scalar.activation(p1[:, :, 0:qn], s_ps[:, :, 0:qn], Exp)
                        p2 = p2pool.tile([128, 3, 512], BF16, tag="p2")
                        nc.vector.tensor_mul(
                            p2[:, :, 0:qn], p1[:, :, 0:qn],
                            s["pats"][:, :, q0 : q0 + qn],
                        )
                        st["pend"].append((ci, q0, qn, o_ps, g, p2))

                def finish():
                    st["g"] += PV_LAG
                    flush(None, 0)
                    st["g"] += RB_LAG + NG
                    flush(None, 0)

                return chunk, finish

            # ---- schedule ----
            ph1_load(0)
            ph1_load(1)
            ph1_tabs(0, 0, 5)
            ph1_relw_tail(0, 0)
            ph1_relw_tail(0, 1)
            chunk0, finish0 = make_main(0)
            chunk1, finish1 = make_main(1)
            chunk0(0)
            ph1_tabs(1, 0, 3)
            chunk0(1)
            ph1_tabs(1, 3, 5)
            chunk0(2)
            ph1_relw_tail(1, 0)
            ph1_relw_tail(1, 1)
            chunk0(3)
            chunk0(4)
            finish0()
            for ci in range(5):
                chunk1(ci)
            finish1()

    nc.compile()
    return nc


def _get_nc():
    global _NC
    if _NC is None:
        _NC = _build_nc()
    return _NC


def _host_prep(q, k, v, rel_pos_h, rel_pos_w):
    q2 = np.asarray(q, np.float32).reshape(HW, NH * DH)
    k2 = np.asarray(k, np.float32).reshape(HW, NH * DH)
    v2 = np.asarray(v, np.float32).reshape(HW, NH * DH)
    rph = np.asarray(rel_pos_h, np.float32)
    rpw = np.asarray(rel_pos_w, np.float32)

    ar = np.arange(48)
    # reversed rel tables, x8 cancels the 0.125 q scale
    rhv = np.ascontiguousarray((8.0 * rph[::-1]).T).astype(BF)   # (64, 95)
    rwv = np.ascontiguousarray((8.0 * rpw[::-1]).T).astype(BF)
    kk = np.arange(HW)
    eh = np.zeros((64, HW), np.float32)
    eh[:48] = kk[None, :] // 48 == ar[:, None]
    eh = eh.astype(BF)

    onecol = np.ones((HW, 1), np.float32)
    in_maps = []
    for c in range(N_CORES):
        sl = slice(c * 128, (c + 1) * 128)
        qs = (q2[:, sl].T * 0.125).astype(BF)                    # (128, HW)
        qw = np.ascontiguousarray(
            qs.reshape(128, 48, 48).transpose(0, 2, 1)
        ).reshape(128, HW)                                       # w-major cols
        ks = k2[:, sl].T.astype(BF)
        vparts = []
        for hh in range(HPC):
            vh = v2[:, c * 128 + hh * 64 : c * 128 + (hh + 1) * 64]
            va = np.concatenate([vh, onecol], axis=1)            # (HW, 65)
            vparts.append(va.reshape(KT, 128, 65).transpose(1, 0, 2).reshape(128, KT * 65))
        v_til = np.concatenate(vparts, axis=1).astype(BF)        # (128, 2*18*65)
        in_maps.append(
            dict(q_t=qs, qw_t=qw, k_t=ks, v_til=v_til, rhv=rhv, rwv=rwv, eh=eh)
        )
    return in_maps


def _assemble(results):
    cols = [np.asarray(r["out_t"], np.float32).T for r in results]  # (HW, 128) each
    return np.concatenate(cols, axis=1).reshape(1, H, W, NH * DH)


def kernel(q, k, v, rel_pos_h, rel_pos_w):
    nc = _get_nc()
    in_maps = _host_prep(q, k, v, rel_pos_h, rel_pos_w)
    res = run_bass_kernel_spmd(nc, in_maps, core_ids=list(range(N_CORES)))
    return _assemble(res.results)


# revision 26
# speedup vs baseline: 1.1366x; 1.1249x over previous
"""Trainium2 Bass kernel for SAM-style decomposed rel-pos attention (v3).

Problem: B=1, HW=2304 (48x48), NH=16 heads, DH=64, D=1024, f32 in/out.
  attn = softmax(q*scale @ k^T + rel_h[qh,kh] + rel_w[qw,kw]); out = attn @ v

Strategy (8 NeuronCores, SPMD): 2 heads per core. Key ideas:
- rel_h is folded into the single score matmul per k-tile (one-hot Eh rows
  + K^T stacked as the stationary operand; gathered rel_h^T rows + Q^T as
  the moving operand) -> 18 score matmuls per (head, chunk) instead of 36.
- rel_w is applied MULTIPLICATIVELY after exp: P = exp(S_qk+relh) * Ew
  where Ew[k,q] = exp(rel_w^T[kw(k), q]). kw(k) is periodic with period 48
  and 128 = 2*48 + 32, so only 3 row-rotations (offsets 0/32/16 = kt mod 3)
  of exp_relw exist -> a [128, 3, HW] "patterns" tile serves every k-tile
  triple via one DVE tensor_mul per 3-k-tile group.
- Diagonal gathers (rel tables are banded) are ONE DMA each via a DRAM
  roundtrip: T1 tables stored to scratch DRAM, re-loaded with a 3D access
  pattern whose middle dim strides -2256 (one row up, 48 cols right).
- exp on ScalarE in [128, 3, qn] groups from PSUM; PV matmuls run PV_LAG
  groups behind the score matmuls; the reciprocal-broadcast matmul of each
  chunk's epilogue is deferred further so the in-order PE stream never
  waits -> PE_HAM un-throttles the clock 1.2 -> 2.4 GHz.
- Head 1's prep (table matmuls, stores, gathers, exp, patterns) is
  interleaved into head 0's main loop at chunk boundaries so the PE and
  DMA rings stay busy; head 0's prep is the only exposed startup.
- Softmax denominator: ones-column in V_aug -> row 64 of the PV output;
  reciprocal on a [128, 4] transposed view (two tiny DMAs) since a
  [1, 512] single-partition reciprocal wastes 127 DVE lanes.
"""

import sys

sys.path.insert(0, "/opt/trn_rl_repo")

import numpy as np
import ml_dtypes

from concourse import bacc, mybir, tile
from concourse.tile import add_dep_helper
from concourse.bass_utils import run_bass_kernel_spmd

BF16 = mybir.dt.bfloat16
F32 = mybir.dt.float32
BF = ml_dtypes.bfloat16

H = 48
W = 48
HW = H * W          # 2304
DH = 64
NH = 16
N_CORES = 8
HPC = 2             # heads per core
KT = HW // 128      # 18 k tiles
QCHUNKS = [(0, 512), (512, 512), (1024, 512), (1536, 512), (2048, 256)]
NG = KT // 3        # 6 groups of 3 k-tiles
PV_LAG = 4          # PV runs this many groups behind the score matmuls
RB_LAG = 7          # reciprocal-broadcast matmul deferral (groups)

_NC = None


def _build_nc():
    nc = bacc.Bacc(None, target_bir_lowering=False)

    lhs_p = nc.dram_tensor("lhs_p", [128, HPC * HW], BF16, kind="ExternalInput")
    rq_p = nc.dram_tensor("rq_p", [128, HPC * 2 * HW], BF16, kind="ExternalInput")
    v_til = nc.dram_tensor("v_til", [128, HPC * KT * 65], BF16, kind="ExternalInput")
    rhv = nc.dram_tensor("rhv", [64, 95], BF16, kind="ExternalInput")
    rwv = nc.dram_tensor("rwv", [64, 95], BF16, kind="ExternalInput")
    out_t = nc.dram_tensor("out_t", [128, HW], F32, kind="ExternalOutput")
    t1d = [nc.dram_tensor(f"t1d{h}", [95, HW], BF16, kind="Internal") for h in range(HPC)]
    t2d = [nc.dram_tensor(f"t2d{h}", [95, HW], BF16, kind="Internal") for h in range(HPC)]

    Exp = mybir.ActivationFunctionType.Exp

    with tile.TileContext(nc) as tc:
        with (
            tc.tile_pool(name="const", bufs=1) as cpool,
            tc.tile_pool(name="stack", bufs=2) as spool,
            tc.tile_pool(name="p1t", bufs=3) as p1pool,
            tc.tile_pool(name="p2t", bufs=5) as p2pool,
            tc.tile_pool(name="epil", bufs=3) as epool,
            tc.tile_pool(name="ps_s", bufs=2, space="PSUM") as ps_s,
            tc.tile_pool(name="ps_o", bufs=2, space="PSUM") as ps_o,
        ):
            rhv_sb = cpool.tile([128, 95], BF16, tag="rhv")
            rwv_sb = cpool.tile([128, 95], BF16, tag="rwv")
            ones1 = cpool.tile([1, 64], BF16, tag="ones1")
            nc.sync.dma_start(rhv_sb[64:128, :], rhv[:, :])
            nc.sync.dma_start(rwv_sb[64:128, :], rwv[:, :])
            nc.gpsimd.memset(ones1[:], 1.0)

            # per-head state dicts
            hs = [dict() for _ in range(HPC)]

            def ph1_load(hh):
                dmae = nc.sync if hh == 0 else nc.gpsimd
                c0, c1 = hh * 64, (hh + 1) * 64
                s = hs[hh]
                s["c0"], s["c1"] = c0, c1
                s["lhsT"] = spool.tile([128, HW], BF16, tag="lhsT", name=f"lhsT{hh}")
                # rq[:, 0, :] = score moving stack (relh rows land in 0:48,
                # zeros 48:64, Q^T 64:128); rq[:, 1, :] = w-major Q^T
                s["rq"] = spool.tile([128, 2, HW], BF16, tag="rq", name=f"rq{hh}")
                s["vt"] = spool.tile([128, KT * 65], BF16, tag="vt", name=f"vt{hh}")
                s["t12"] = spool.tile([95, 2, HW], BF16, tag="t12", name=f"t12{hh}")
                s["relw"] = spool.tile([48, HW], BF16, tag="relw", name=f"relw{hh}")
                s["expw"] = spool.tile([48, HW], BF16, tag="expw", name=f"expw{hh}")
                s["pats"] = spool.tile([128, 3, HW], BF16, tag="pats", name=f"pats{hh}")
                for (l0, ln) in ((0, 768), (768, 768), (1536, 768)):
                    dmae.dma_start(
                        s["rq"][:, :, l0 : l0 + ln],
                        rq_p[:, 2 * hh * HW : 2 * (hh + 1) * HW].rearrange(
                            "p (a b) -> p a b", b=HW
                        )[:, :, l0 : l0 + ln],
                    )
                dmae.dma_start(
                    s["lhsT"][:, :], lhs_p[:, hh * HW : (hh + 1) * HW]
                )
                dmae.dma_start(s["vt"][:, :], v_til[:, hh * KT * 65 : (hh + 1) * KT * 65])

            # gather piece ranges: piece c covers blocks [lo, hi) whose cols
            # fit inside table-store chunks 0..c (512 cols each)
            PIECES = [(0, 10), (10, 21), (21, 32), (32, 42), (42, 48)]

            def ph1_tabs(hh, lo, hi):
                # per chunk: T2w (w-major rel_w table) and T1h (rel_h table)
                # matmuls -> one fused cast -> chunked DRAM stores (ring A)
                # -> gather pieces (ring B: never stuck behind a store).
                # t12[:, 0, :] = T2w, t12[:, 1, :] = T1h.
                s = hs[hh]
                ringA = nc.sync if hh == 0 else nc.gpsimd
                ringB = nc.gpsimd if hh == 0 else nc.sync
                for ci in range(lo, hi):
                    (q0, qn) = QCHUNKS[ci]
                    tp = ps_s.tile([128, 3, 512], F32, tag="s", name=f"tp{hh}_{ci}")
                    nc.tensor.matmul(
                        tp[0:95, 0, 0:qn], rwv_sb[64:128, :],
                        s["rq"][64:128, 1, q0 : q0 + qn], start=True, stop=True,
                    )
                    nc.tensor.matmul(
                        tp[0:95, 1, 0:qn], rhv_sb[64:128, :],
                        s["rq"][64:128, 0, q0 : q0 + qn], start=True, stop=True,
                    )
                    nc.vector.tensor_copy(
                        s["t12"][:, :, q0 : q0 + qn], tp[0:95, 0:2, 0:qn]
                    )
                    for tbl, dram in ((0, t2d[hh]), (1, t1d[hh])):
                        ringA.dma_start(
                            dram[0:95, q0 : q0 + qn], s["t12"][0:95, tbl, q0 : q0 + qn]
                        )
                    # gather pieces (diagonal re-load)
                    (ba, bb) = PIECES[ci]
                    nbl = bb - ba
                    # rel_h: rq[j, 0, (h,w)] = T1h[47-h+j, 48h+w], h in [ba, bb)
                    dsth = s["rq"][0:48, 0, 48 * ba : 48 * bb].rearrange(
                        "p (h w) -> p h w", w=48
                    )
                    srch = t1d[hh][47 - ba : 95 - ba, 48 * ba : 48 * bb].rearrange(
                        "j (h w) -> j h w", w=48
                    )
                    srch.ap[1] = [-2256, nbl]
                    ringB.dma_start(dsth, srch)
                    # rel_w (w-major): relw[j, (w,h)] = T2w[47-w+j, 48w+h]
                    dstw = s["relw"][0:48, 48 * ba : 48 * bb].rearrange(
                        "p (w h) -> p w h", h=48
                    )
                    srcw = t2d[hh][47 - ba : 95 - ba, 48 * ba : 48 * bb].rearrange(
                        "j (w h) -> j w h", h=48
                    )
                    srcw.ap[1] = [-2256, nbl]
                    ringB.dma_start(dstw, srcw)

            def ph1_relw_tail(hh, half):
                # exp with an un-permuting (w-major -> q-major) input AP
                # half=None: full width in one pass (fewer DMAs; for the
                # latency-hidden head)
                s = hs[hh]
                if half is None:
                    h0, co, cn = 0, 0, HW
                    nhb = 48
                else:
                    h0 = half * 24  # h-blocks 0:24/24:48 -> cols 0:1152/1152:2304
                    co, cn = h0 * 48, 1152
                    nhb = 24
                nc.scalar.activation(
                    s["expw"][:, co : co + cn].rearrange("p (h w) -> p h w", w=48),
                    s["relw"][:, :].rearrange("p (w h) -> p h w", w=48)[:, h0 : h0 + nhb, :],
                    Exp,
                )
                ndma = 0
                for j, off in enumerate((0, 32, 16)):
                    p = 0
                    while p < 128:
                        r0 = (p + off) % 48
                        n = min(48 - r0, 128 - p)
                        eng = (nc.sync, nc.gpsimd)[(hh + ndma) % 2]
                        eng.dma_start(
                            s["pats"][p : p + n, j, co : co + cn],
                            s["expw"][r0 : r0 + n, co : co + cn],
                        )
                        p += n
                        ndma += 1

            # ---- main loop machinery (per head) ----
            def make_main(hh):
                s = hs[hh]
                st = {"pend": [], "epiB": [], "g": 0}

                def epilogue_a(ci, q0, qn, o_ps):
                    den = epool.tile([1, 512], F32, tag="den", name=f"den{hh}_{ci}")
                    nc.vector.tensor_copy(den[0:1, 0:qn], o_ps[64:65, 0:qn])
                    dmae = nc.sync if ci % 2 == 0 else nc.gpsimd
                    dent = epool.tile([128, 4], F32, tag="dent", name=f"dent{hh}_{ci}")
                    nr = qn // 128
                    dmae.dma_start(dent[0:128, 0:nr], den[0:1, 0:qn])
                    rect = epool.tile([128, 4], F32, tag="rect", name=f"rect{hh}_{ci}")
                    nc.vector.reciprocal(rect[0:128, 0:nr], dent[0:128, 0:nr])
                    recb = epool.tile([128, 4], BF16, tag="recb", name=f"recb{hh}_{ci}")
                    nc.vector.tensor_copy(recb[0:128, 0:nr], rect[0:128, 0:nr])
                    recr = epool.tile([1, 512], BF16, tag="recr", name=f"recr{hh}_{ci}")
                    dmae.dma_start(recr[0:1, 0:qn], recb[0:128, 0:nr])
                    return recr

                def epilogue_b(ci, q0, qn, o_ps, recr):
                    rb_ps = ps_s.tile([128, 3, 512], F32, tag="s", name=f"rb{hh}_{ci}")
                    nc.tensor.matmul(
                        rb_ps[0:64, 0, 0:qn], ones1[:], recr[0:1, 0:qn],
                        start=True, stop=True,
                    )
                    rb_sb = epool.tile([64, 512], F32, tag="rb_sb", name=f"rbs{hh}_{ci}")
                    nc.vector.tensor_copy(rb_sb[:, 0:qn], rb_ps[0:64, 0, 0:qn])
                    ot = epool.tile([64, 512], F32, tag="ot", name=f"ot{hh}_{ci}")
                    nc.vector.tensor_mul(
                        ot[:, 0:qn], o_ps[0:64, 0:qn], rb_sb[:, 0:qn]
                    )
                    nc.sync.dma_start(out_t[s["c0"] : s["c1"], q0 : q0 + qn], ot[:, 0:qn])

                def flush(last_mm, keep):
                    while len(st["pend"]) > keep:
                        (ci, q0, qn, o_ps, g, p2) = st["pend"].pop(0)
                        for j in range(3):
                            kt = 3 * g + j
                            pv = nc.tensor.matmul(
                                o_ps[0:65, 0:qn],
                                s["vt"][:, kt * 65 : (kt + 1) * 65],
                                p2[:, j, 0:qn],
                                start=(kt == 0), stop=(kt == KT - 1),
                            )
                            if last_mm is not None:
                                add_dep_helper(pv.ins, last_mm.ins, sync=False,
                                               reason="pv after score mms")
                        if g == NG - 1:
                            recr = epilogue_a(ci, q0, qn, o_ps)
                            st["epiB"].append((ci, q0, qn, o_ps, recr, st["g"] + RB_LAG))
                    while st["epiB"] and st["epiB"][0][5] <= st["g"]:
                        (ci, q0, qn, o_ps, recr, _) = st["epiB"].pop(0)
                        epilogue_b(ci, q0, qn, o_ps, recr)

                def chunk(ci):
                    (q0, qn) = QCHUNKS[ci]
                    o_ps = ps_o.tile([65, 512], F32, tag="o", name=f"o{hh}_{ci}")
                    for g in range(NG):
                        s_ps = ps_s.tile([128, 3, 512], F32, tag="s",
                                         name=f"s{hh}_{ci}_{g}")
                        last_mm = None
                        for j in range(3):
                            kt = 3 * g + j
                            last_mm = nc.tensor.matmul(
                                s_ps[:, j, 0:qn],
                                s["lhsT"][:, kt * 128 : (kt + 1) * 128],
                                s["rq"][:, 0, q0 : q0 + qn],
                                start=True, stop=True,
                            )
                        st["g"] += 1
                        flush(last_mm, PV_LAG - 1)
                        p1 = p1pool.tile([128, 3, 512], BF16, tag="p1")
                        nc.scalar.activation(p1[:, :, 0:qn], s_ps[:, :, 0:qn], Exp)
                        p2 = p2pool.tile([128, 3, 512], BF16, tag="p2")
                        nc.vector.tensor_mul(
                            p2[:, :, 0:qn], p1[:, :, 0:qn],
                            s["pats"][:, :, q0 : q0 + qn],
                        )
                        st["pend"].append((ci, q0, qn, o_ps, g, p2))

                def finish_pvs():
                    st["g"] += PV_LAG
                    flush(None, 0)

                def drain(n=1):
                    for _ in range(n):
                        if st["epiB"]:
                            (ci, q0, qn, o_ps, recr, _) = st["epiB"].pop(0)
                            epilogue_b(ci, q0, qn, o_ps, recr)

                return chunk, finish_pvs, drain

            # ---- schedule ----
            ph1_load(0)
            ph1_load(1)
            ph1_tabs(0, 0, 5)
            ph1_relw_tail(0, 0)
            ph1_relw_tail(0, 1)
            chunk0, finish_pvs0, drain0 = make_main(0)
            chunk1, finish_pvs1, drain1 = make_main(1)
            chunk0(0)
            ph1_tabs(1, 0, 3)
            chunk0(1)
            ph1_tabs(1, 3, 5)
            chunk0(2)
            ph1_relw_tail(1, 0)
            ph1_relw_tail(1, 1)
            chunk0(3)
            chunk0(4)
            finish_pvs0()
            chunk1(0)
            drain0(2)
            chunk1(1)
            drain0(2)
            chunk1(2)
            drain0(2)
            chunk1(3)
            chunk1(4)
            finish_pvs1()
            drain1(5)

    nc.compile()
    return nc


def _get_nc():
    global _NC
    if _NC is None:
        _NC = _build_nc()
    return _NC


def _host_prep(q, k, v, rel_pos_h, rel_pos_w):
    q2 = np.asarray(q, np.float32).reshape(HW, NH * DH)
    k2 = np.asarray(k, np.float32).reshape(HW, NH * DH)
    v2 = np.asarray(v, np.float32).reshape(HW, NH * DH)
    rph = np.asarray(rel_pos_h, np.float32)
    rpw = np.asarray(rel_pos_w, np.float32)

    ar = np.arange(48)
    # reversed rel tables, x8 cancels the 0.125 q scale
    rhv = np.ascontiguousarray((8.0 * rph[::-1]).T).astype(BF)   # (64, 95)
    rwv = np.ascontiguousarray((8.0 * rpw[::-1]).T).astype(BF)
    kk = np.arange(HW)
    eh = np.zeros((64, HW), np.float32)
    eh[:48] = kk[None, :] // 48 == ar[:, None]
    eh = eh.astype(BF)

    onecol = np.ones((HW, 1), np.float32)
    in_maps = []
    for c in range(N_CORES):
        sl = slice(c * 128, (c + 1) * 128)
        qs = (q2[:, sl].T * 0.125).astype(BF)                    # (128, HW)
        qw = np.ascontiguousarray(
            qs.reshape(128, 48, 48).transpose(0, 2, 1)
        ).reshape(128, HW)                                       # w-major cols
        ks = k2[:, sl].T.astype(BF)
        lhs_p = np.zeros((128, HPC, HW), BF)
        rq_p = np.zeros((128, HPC, 2, HW), BF)
        vparts = []
        for hh in range(HPC):
            r0, r1 = hh * 64, (hh + 1) * 64
            lhs_p[0:64, hh, :] = eh
            lhs_p[64:128, hh, :] = ks[r0:r1]
            rq_p[64:128, hh, 0, :] = qs[r0:r1]
            rq_p[64:128, hh, 1, :] = qw[r0:r1]
            vh = v2[:, c * 128 + hh * 64 : c * 128 + (hh + 1) * 64]
            va = np.concatenate([vh, onecol], axis=1)            # (HW, 65)
            vparts.append(va.reshape(KT, 128, 65).transpose(1, 0, 2).reshape(128, KT * 65))
        v_til = np.concatenate(vparts, axis=1).astype(BF)        # (128, 2*18*65)
        in_maps.append(
            dict(
                lhs_p=lhs_p.reshape(128, HPC * HW),
                rq_p=rq_p.reshape(128, HPC * 2 * HW),
                v_til=v_til, rhv=rhv, rwv=rwv,
            )
        )
    return in_maps


def _assemble(results):
    cols = [np.asarray(r["out_t"], np.float32).T for r in results]  # (HW, 128) each
    return np.concatenate(cols, axis=1).reshape(1, H, W, NH * DH)


def kernel(q, k, v, rel_pos_h, rel_pos_w):
    nc = _get_nc()
    in_maps = _host_prep(q, k, v, rel_pos_h, rel_pos_w)
    res = run_bass_kernel_spmd(nc, in_maps, core_ids=list(range(N_CORES)))
    return _assemble(res.results)


# revision 27
# speedup vs baseline: 1.1476x; 1.0097x over previous
"""Trainium2 Bass kernel for SAM-style decomposed rel-pos attention (v3).

Problem: B=1, HW=2304 (48x48), NH=16 heads, DH=64, D=1024, f32 in/out.
  attn = softmax(q*scale @ k^T + rel_h[qh,kh] + rel_w[qw,kw]); out = attn @ v

Strategy (8 NeuronCores, SPMD): 2 heads per core. Key ideas:
- rel_h is folded into the single score matmul per k-tile (one-hot Eh rows
  + K^T stacked as the stationary operand; gathered rel_h^T rows + Q^T as
  the moving operand) -> 18 score matmuls per (head, chunk) instead of 36.
- rel_w is applied MULTIPLICATIVELY after exp: P = exp(S_qk+relh) * Ew
  where Ew[k,q] = exp(rel_w^T[kw(k), q]). kw(k) is periodic with period 48
  and 128 = 2*48 + 32, so only 3 row-rotations (offsets 0/32/16 = kt mod 3)
  of exp_relw exist -> a [128, 3, HW] "patterns" tile serves every k-tile
  triple via one DVE tensor_mul per 3-k-tile group.
- Diagonal gathers (rel tables are banded) are ONE DMA each via a DRAM
  roundtrip: T1 tables stored to scratch DRAM, re-loaded with a 3D access
  pattern whose middle dim strides -2256 (one row up, 48 cols right).
- exp on ScalarE in [128, 3, qn] groups from PSUM; PV matmuls run PV_LAG
  groups behind the score matmuls; the reciprocal-broadcast matmul of each
  chunk's epilogue is deferred further so the in-order PE stream never
  waits -> PE_HAM un-throttles the clock 1.2 -> 2.4 GHz.
- Head 1's prep (table matmuls, stores, gathers, exp, patterns) is
  interleaved into head 0's main loop at chunk boundaries so the PE and
  DMA rings stay busy; head 0's prep is the only exposed startup.
- Softmax denominator: ones-column in V_aug -> row 64 of the PV output;
  reciprocal on a [128, 4] transposed view (two tiny DMAs) since a
  [1, 512] single-partition reciprocal wastes 127 DVE lanes.
"""

import sys

sys.path.insert(0, "/opt/trn_rl_repo")

import numpy as np
import ml_dtypes

from concourse import bacc, mybir, tile
from concourse.tile import add_dep_helper
from concourse.bass_utils import run_bass_kernel_spmd

BF16 = mybir.dt.bfloat16
F32 = mybir.dt.float32
BF = ml_dtypes.bfloat16

H = 48
W = 48
HW = H * W          # 2304
DH = 64
NH = 16
N_CORES = 8
HPC = 2             # heads per core
KT = HW // 128      # 18 k tiles
QCHUNKS = [(0, 512), (512, 512), (1024, 512), (1536, 512), (2048, 256)]
NG = KT // 3        # 6 groups of 3 k-tiles
PV_LAG = 4          # PV runs this many groups behind the score matmuls
RB_LAG = 7          # reciprocal-broadcast matmul deferral (groups)

_NC = None


def _build_nc():
    nc = bacc.Bacc(None, target_bir_lowering=False)

    lhs_p = nc.dram_tensor("lhs_p", [128, HPC * HW], BF16, kind="ExternalInput")
    rq_p = nc.dram_tensor("rq_p", [128, HPC * 2 * HW], BF16, kind="ExternalInput")
    v_til = nc.dram_tensor("v_til", [128, HPC * KT * 65], BF16, kind="ExternalInput")
    rhv = nc.dram_tensor("rhv", [64, 95], BF16, kind="ExternalInput")
    rwv = nc.dram_tensor("rwv", [64, 95], BF16, kind="ExternalInput")
    out_t = nc.dram_tensor("out_t", [128, HW], F32, kind="ExternalOutput")
    t1d = [nc.dram_tensor(f"t1d{h}", [95, HW], BF16, kind="Internal") for h in range(HPC)]
    t2d = [nc.dram_tensor(f"t2d{h}", [95, HW], BF16, kind="Internal") for h in range(HPC)]

    Exp = mybir.ActivationFunctionType.Exp

    with tile.TileContext(nc) as tc:
        with (
            tc.tile_pool(name="const", bufs=1) as cpool,
            tc.tile_pool(name="stack", bufs=2) as spool,
            tc.tile_pool(name="p1t", bufs=4) as p1pool,
            tc.tile_pool(name="p2t", bufs=5) as p2pool,
            tc.tile_pool(name="epil", bufs=3) as epool,
            tc.tile_pool(name="ps_s", bufs=2, space="PSUM") as ps_s,
            tc.tile_pool(name="ps_o", bufs=2, space="PSUM") as ps_o,
        ):
            rhv_sb = cpool.tile([128, 95], BF16, tag="rhv")
            rwv_sb = cpool.tile([128, 95], BF16, tag="rwv")
            ones1 = cpool.tile([1, 64], BF16, tag="ones1")
            nc.sync.dma_start(rhv_sb[64:128, :], rhv[:, :])
            nc.sync.dma_start(rwv_sb[64:128, :], rwv[:, :])
            nc.gpsimd.memset(ones1[:], 1.0)

            # per-head state dicts
            hs = [dict() for _ in range(HPC)]

            def ph1_load(hh):
                dmae = nc.sync if hh == 0 else nc.gpsimd
                c0, c1 = hh * 64, (hh + 1) * 64
                s = hs[hh]
                s["c0"], s["c1"] = c0, c1
                s["lhsT"] = spool.tile([128, HW], BF16, tag="lhsT", name=f"lhsT{hh}")
                # rq[:, 0, :] = score moving stack (relh rows land in 0:48,
                # zeros 48:64, Q^T 64:128); rq[:, 1, :] = w-major Q^T
                s["rq"] = spool.tile([128, 2, HW], BF16, tag="rq", name=f"rq{hh}")
                s["vt"] = spool.tile([128, KT * 65], BF16, tag="vt", name=f"vt{hh}")
                s["t12"] = spool.tile([95, 2, HW], BF16, tag="t12", name=f"t12{hh}")
                s["relw"] = spool.tile([48, HW], BF16, tag="relw", name=f"relw{hh}")
                s["expw"] = spool.tile([48, HW], BF16, tag="expw", name=f"expw{hh}")
                s["pats"] = spool.tile([128, 3, HW], BF16, tag="pats", name=f"pats{hh}")
                for (l0, ln) in ((0, 768), (768, 768), (1536, 768)):
                    dmae.dma_start(
                        s["rq"][:, :, l0 : l0 + ln],
                        rq_p[:, 2 * hh * HW : 2 * (hh + 1) * HW].rearrange(
                            "p (a b) -> p a b", b=HW
                        )[:, :, l0 : l0 + ln],
                    )
                dmae.dma_start(
                    s["lhsT"][:, :], lhs_p[:, hh * HW : (hh + 1) * HW]
                )
                dmae.dma_start(s["vt"][:, :], v_til[:, hh * KT * 65 : (hh + 1) * KT * 65])

            # gather piece ranges: piece c covers blocks [lo, hi) whose cols
            # fit inside table-store chunks 0..c (512 cols each)
            PIECES = [(0, 10), (10, 21), (21, 32), (32, 42), (42, 48)]

            def ph1_tabs(hh, lo, hi):
                # per chunk: T2w (w-major rel_w table) and T1h (rel_h table)
                # matmuls -> one fused cast -> chunked DRAM stores (ring A)
                # -> gather pieces (ring B: never stuck behind a store).
                # t12[:, 0, :] = T2w, t12[:, 1, :] = T1h.
                s = hs[hh]
                ringA = nc.sync if hh == 0 else nc.gpsimd
                ringB = nc.gpsimd if hh == 0 else nc.sync
                for ci in range(lo, hi):
                    (q0, qn) = QCHUNKS[ci]
                    tp = ps_s.tile([128, 3, 512], F32, tag="s", name=f"tp{hh}_{ci}")
                    nc.tensor.matmul(
                        tp[0:95, 0, 0:qn], rwv_sb[64:128, :],
                        s["rq"][64:128, 1, q0 : q0 + qn], start=True, stop=True,
                    )
                    nc.tensor.matmul(
                        tp[0:95, 1, 0:qn], rhv_sb[64:128, :],
                        s["rq"][64:128, 0, q0 : q0 + qn], start=True, stop=True,
                    )
                    nc.vector.tensor_copy(
                        s["t12"][:, :, q0 : q0 + qn], tp[0:95, 0:2, 0:qn]
                    )
                    for tbl, dram in ((0, t2d[hh]), (1, t1d[hh])):
                        ringA.dma_start(
                            dram[0:95, q0 : q0 + qn], s["t12"][0:95, tbl, q0 : q0 + qn]
                        )
                    # gather pieces (diagonal re-load)
                    (ba, bb) = PIECES[ci]
                    nbl = bb - ba
                    # rel_h: rq[j, 0, (h,w)] = T1h[47-h+j, 48h+w], h in [ba, bb)
                    dsth = s["rq"][0:48, 0, 48 * ba : 48 * bb].rearrange(
                        "p (h w) -> p h w", w=48
                    )
                    srch = t1d[hh][47 - ba : 95 - ba, 48 * ba : 48 * bb].rearrange(
                        "j (h w) -> j h w", w=48
                    )
                    srch.ap[1] = [-2256, nbl]
                    ringB.dma_start(dsth, srch)
                    # rel_w (w-major): relw[j, (w,h)] = T2w[47-w+j, 48w+h]
                    dstw = s["relw"][0:48, 48 * ba : 48 * bb].rearrange(
                        "p (w h) -> p w h", h=48
                    )
                    srcw = t2d[hh][47 - ba : 95 - ba, 48 * ba : 48 * bb].rearrange(
                        "j (w h) -> j w h", h=48
                    )
                    srcw.ap[1] = [-2256, nbl]
                    ringB.dma_start(dstw, srcw)

            def ph1_relw_tail(hh, half):
                # exp with an un-permuting (w-major -> q-major) input AP
                # half=None: full width in one pass (fewer DMAs; for the
                # latency-hidden head)
                s = hs[hh]
                if half is None:
                    h0, co, cn = 0, 0, HW
                    nhb = 48
                else:
                    h0 = half * 24  # h-blocks 0:24/24:48 -> cols 0:1152/1152:2304
                    co, cn = h0 * 48, 1152
                    nhb = 24
                nc.scalar.activation(
                    s["expw"][:, co : co + cn].rearrange("p (h w) -> p h w", w=48),
                    s["relw"][:, :].rearrange("p (w h) -> p h w", w=48)[:, h0 : h0 + nhb, :],
                    Exp,
                )
                ndma = 0
                for j, off in enumerate((0, 32, 16)):
                    p = 0
                    while p < 128:
                        r0 = (p + off) % 48
                        n = min(48 - r0, 128 - p)
                        eng = (nc.sync, nc.gpsimd)[(hh + ndma) % 2]
                        eng.dma_start(
                            s["pats"][p : p + n, j, co : co + cn],
                            s["expw"][r0 : r0 + n, co : co + cn],
                        )
                        p += n
                        ndma += 1

            # ---- main loop machinery (per head) ----
            def make_main(hh):
                s = hs[hh]
                st = {"pend": [], "epiB": [], "g": 0}

                def epilogue_a(ci, q0, qn, o_ps):
                    den = epool.tile([1, 512], F32, tag="den", name=f"den{hh}_{ci}")
                    nc.vector.tensor_copy(den[0:1, 0:qn], o_ps[64:65, 0:qn])
                    dmae = nc.sync if ci % 2 == 0 else nc.gpsimd
                    dent = epool.tile([128, 4], F32, tag="dent", name=f"dent{hh}_{ci}")
                    nr = qn // 128
                    dmae.dma_start(dent[0:128, 0:nr], den[0:1, 0:qn])
                    rect = epool.tile([128, 4], F32, tag="rect", name=f"rect{hh}_{ci}")
                    nc.vector.reciprocal(rect[0:128, 0:nr], dent[0:128, 0:nr])
                    recb = epool.tile([128, 4], BF16, tag="recb", name=f"recb{hh}_{ci}")
                    nc.vector.tensor_copy(recb[0:128, 0:nr], rect[0:128, 0:nr])
                    recr = epool.tile([1, 512], BF16, tag="recr", name=f"recr{hh}_{ci}")
                    dmae.dma_start(recr[0:1, 0:qn], recb[0:128, 0:nr])
                    return recr

                def epilogue_b(ci, q0, qn, o_ps, recr):
                    rb_ps = ps_s.tile([128, 3, 512], F32, tag="s", name=f"rb{hh}_{ci}")
                    nc.tensor.matmul(
                        rb_ps[0:64, 0, 0:qn], ones1[:], recr[0:1, 0:qn],
                        start=True, stop=True,
                    )
                    rb_sb = epool.tile([64, 512], F32, tag="rb_sb", name=f"rbs{hh}_{ci}")
                    nc.vector.tensor_copy(rb_sb[:, 0:qn], rb_ps[0:64, 0, 0:qn])
                    ot = epool.tile([64, 512], F32, tag="ot", name=f"ot{hh}_{ci}")
                    nc.vector.tensor_mul(
                        ot[:, 0:qn], o_ps[0:64, 0:qn], rb_sb[:, 0:qn]
                    )
                    nc.sync.dma_start(out_t[s["c0"] : s["c1"], q0 : q0 + qn], ot[:, 0:qn])

                def flush(last_mm, keep):
                    while len(st["pend"]) > keep:
                        (ci, q0, qn, o_ps, g, p2) = st["pend"].pop(0)
                        for j in range(3):
                            kt = 3 * g + j
                            pv = nc.tensor.matmul(
                                o_ps[0:65, 0:qn],
                                s["vt"][:, kt * 65 : (kt + 1) * 65],
                                p2[:, j, 0:qn],
                                start=(kt == 0), stop=(kt == KT - 1),
                            )
                            if last_mm is not None:
                                add_dep_helper(pv.ins, last_mm.ins, sync=False,
                                               reason="pv after score mms")
                        if g == NG - 1:
                            recr = epilogue_a(ci, q0, qn, o_ps)
                            st["epiB"].append((ci, q0, qn, o_ps, recr, st["g"] + RB_LAG))
                    while st["epiB"] and st["epiB"][0][5] <= st["g"]:
                        (ci, q0, qn, o_ps, recr, _) = st["epiB"].pop(0)
                        epilogue_b(ci, q0, qn, o_ps, recr)

                def chunk(ci):
                    (q0, qn) = QCHUNKS[ci]
                    o_ps = ps_o.tile([65, 512], F32, tag="o", name=f"o{hh}_{ci}")
                    for g in range(NG):
                        s_ps = ps_s.tile([128, 3, 512], F32, tag="s",
                                         name=f"s{hh}_{ci}_{g}")
                        last_mm = None
                        for j in range(3):
                            kt = 3 * g + j
                            last_mm = nc.tensor.matmul(
                                s_ps[:, j, 0:qn],
                                s["lhsT"][:, kt * 128 : (kt + 1) * 128],
                                s["rq"][:, 0, q0 : q0 + qn],
                                start=True, stop=True,
                            )
                        st["g"] += 1
                        flush(last_mm, PV_LAG - 1)
                        p1 = p1pool.tile([128, 3, 512], BF16, tag="p1")
                        nc.scalar.activation(p1[:, :, 0:qn], s_ps[:, :, 0:qn], Exp)
                        p2 = p2pool.tile([128, 3, 512], BF16, tag="p2")
                        nc.vector.tensor_mul(
                            p2[:, :, 0:qn], p1[:, :, 0:qn],
                            s["pats"][:, :, q0 : q0 + qn],
                        )
                        st["pend"].append((ci, q0, qn, o_ps, g, p2))

                def finish_pvs():
                    st["g"] += PV_LAG
                    flush(None, 0)

                def drain(n=1):
                    for _ in range(n):
                        if st["epiB"]:
                            (ci, q0, qn, o_ps, recr, _) = st["epiB"].pop(0)
                            epilogue_b(ci, q0, qn, o_ps, recr)

                return chunk, finish_pvs, drain

            # ---- schedule ----
            ph1_load(0)
            ph1_load(1)
            ph1_tabs(0, 0, 5)
            ph1_relw_tail(0, 0)
            ph1_relw_tail(0, 1)
            chunk0, finish_pvs0, drain0 = make_main(0)
            chunk1, finish_pvs1, drain1 = make_main(1)
            chunk0(0)
            ph1_tabs(1, 0, 3)
            chunk0(1)
            ph1_tabs(1, 3, 5)
            chunk0(2)
            ph1_relw_tail(1, 0)
            ph1_relw_tail(1, 1)
            chunk0(3)
            chunk0(4)
            finish_pvs0()
            chunk1(0)
            drain0(2)
            chunk1(1)
            drain0(2)
            chunk1(2)
            drain0(2)
            chunk1(3)
            chunk1(4)
            finish_pvs1()
            drain1(5)

    nc.compile()
    return nc


def _get_nc():
    global _NC
    if _NC is None:
        _NC = _build_nc()
    return _NC


def _host_prep(q, k, v, rel_pos_h, rel_pos_w):
    q2 = np.asarray(q, np.float32).reshape(HW, NH * DH)
    k2 = np.asarray(k, np.float32).reshape(HW, NH * DH)
    v2 = np.asarray(v, np.float32).reshape(HW, NH * DH)
    rph = np.asarray(rel_pos_h, np.float32)
    rpw = np.asarray(rel_pos_w, np.float32)

    ar = np.arange(48)
    # reversed rel tables, x8 cancels the 0.125 q scale
    rhv = np.ascontiguousarray((8.0 * rph[::-1]).T).astype(BF)   # (64, 95)
    rwv = np.ascontiguousarray((8.0 * rpw[::-1]).T).astype(BF)
    kk = np.arange(HW)
    eh = np.zeros((64, HW), np.float32)
    eh[:48] = kk[None, :] // 48 == ar[:, None]
    eh = eh.astype(BF)

    onecol = np.ones((HW, 1), np.float32)
    in_maps = []
    for c in range(N_CORES):
        sl = slice(c * 128, (c + 1) * 128)
        qs = (q2[:, sl].T * 0.125).astype(BF)                    # (128, HW)
        qw = np.ascontiguousarray(
            qs.reshape(128, 48, 48).transpose(0, 2, 1)
        ).reshape(128, HW)                                       # w-major cols
        ks = k2[:, sl].T.astype(BF)
        lhs_p = np.zeros((128, HPC, HW), BF)
        rq_p = np.zeros((128, HPC, 2, HW), BF)
        vparts = []
        for hh in range(HPC):
            r0, r1 = hh * 64, (hh + 1) * 64
            lhs_p[0:64, hh, :] = eh
            lhs_p[64:128, hh, :] = ks[r0:r1]
            rq_p[64:128, hh, 0, :] = qs[r0:r1]
            rq_p[64:128, hh, 1, :] = qw[r0:r1]
            vh = v2[:, c * 128 + hh * 64 : c * 128 + (hh + 1) * 64]
            va = np.concatenate([vh, onecol], axis=1)            # (HW, 65)
            vparts.append(va.reshape(KT, 128, 65).transpose(1, 0, 2).reshape(128, KT * 65))
        v_til = np.concatenate(vparts, axis=1).astype(BF)        # (128, 2*18*65)
        in_maps.append(
            dict(
                lhs_p=lhs_p.reshape(128, HPC * HW),
                rq_p=rq_p.reshape(128, HPC * 2 * HW),
                v_til=v_til, rhv=rhv, rwv=rwv,
            )
        )
    return in_maps


def _assemble(results):
    cols = [np.asarray(r["out_t"], np.float32).T for r in results]  # (HW, 128) each
    return np.concatenate(cols, axis=1).reshape(1, H, W, NH * DH)


def kernel(q, k, v, rel_pos_h, rel_pos_w):
    nc = _get_nc()
    in_maps = _host_prep(q, k, v, rel_pos_h, rel_pos_w)
    res = run_bass_kernel_spmd(nc, in_maps, core_ids=list(range(N_CORES)))
    return _assemble(res.results)


# revision 28
# speedup vs baseline: 1.1515x; 1.0034x over previous
"""Trainium2 Bass kernel for SAM-style decomposed rel-pos attention (v3).

Problem: B=1, HW=2304 (48x48), NH=16 heads, DH=64, D=1024, f32 in/out.
  attn = softmax(q*scale @ k^T + rel_h[qh,kh] + rel_w[qw,kw]); out = attn @ v

Strategy (8 NeuronCores, SPMD): 2 heads per core. Key ideas:
- rel_h is folded into the single score matmul per k-tile (one-hot Eh rows
  + K^T stacked as the stationary operand; gathered rel_h^T rows + Q^T as
  the moving operand) -> 18 score matmuls per (head, chunk) instead of 36.
- rel_w is applied MULTIPLICATIVELY after exp: P = exp(S_qk+relh) * Ew
  where Ew[k,q] = exp(rel_w^T[kw(k), q]). kw(k) is periodic with period 48
  and 128 = 2*48 + 32, so only 3 row-rotations (offsets 0/32/16 = kt mod 3)
  of exp_relw exist -> a [128, 3, HW] "patterns" tile serves every k-tile
  triple via one DVE tensor_mul per 3-k-tile group.
- Diagonal gathers (rel tables are banded) are ONE DMA each via a DRAM
  roundtrip: T1 tables stored to scratch DRAM, re-loaded with a 3D access
  pattern whose middle dim strides -2256 (one row up, 48 cols right).
- exp on ScalarE in [128, 3, qn] groups from PSUM; PV matmuls run PV_LAG
  groups behind the score matmuls; the reciprocal-broadcast matmul of each
  chunk's epilogue is deferred further so the in-order PE stream never
  waits -> PE_HAM un-throttles the clock 1.2 -> 2.4 GHz.
- Head 1's prep (table matmuls, stores, gathers, exp, patterns) is
  interleaved into head 0's main loop at chunk boundaries so the PE and
  DMA rings stay busy; head 0's prep is the only exposed startup.
- Softmax denominator: ones-column in V_aug -> row 64 of the PV output;
  reciprocal on a [128, 4] transposed view (two tiny DMAs) since a
  [1, 512] single-partition reciprocal wastes 127 DVE lanes.
"""

import sys

sys.path.insert(0, "/opt/trn_rl_repo")

import numpy as np
import ml_dtypes

from concourse import bacc, mybir, tile
from concourse.tile import add_dep_helper
from concourse.bass_utils import run_bass_kernel_spmd

BF16 = mybir.dt.bfloat16
F32 = mybir.dt.float32
BF = ml_dtypes.bfloat16

H = 48
W = 48
HW = H * W          # 2304
DH = 64
NH = 16
N_CORES = 8
HPC = 2             # heads per core
KT = HW // 128      # 18 k tiles
QCHUNKS = [(0, 512), (512, 512), (1024, 512), (1536, 512), (2048, 256)]
NG = KT // 3        # 6 groups of 3 k-tiles
PV_LAG = 4          # PV runs this many groups behind the score matmuls
RB_LAG = 7          # reciprocal-broadcast matmul deferral (groups)

_NC = None


def _build_nc():
    nc = bacc.Bacc(None, target_bir_lowering=False)

    lhs_p = nc.dram_tensor("lhs_p", [128, HPC * HW], BF16, kind="ExternalInput")
    rq_p = nc.dram_tensor("rq_p", [128, HPC * 2 * HW], BF16, kind="ExternalInput")
    v_til = nc.dram_tensor("v_til", [128, HPC * KT * 65], BF16, kind="ExternalInput")
    rhv = nc.dram_tensor("rhv", [64, 95], BF16, kind="ExternalInput")
    rwv = nc.dram_tensor("rwv", [64, 95], BF16, kind="ExternalInput")
    out_t = nc.dram_tensor("out_t", [128, HW], F32, kind="ExternalOutput")
    t1d = [nc.dram_tensor(f"t1d{h}", [95, HW], BF16, kind="Internal") for h in range(HPC)]
    t2d = [nc.dram_tensor(f"t2d{h}", [95, HW], BF16, kind="Internal") for h in range(HPC)]

    Exp = mybir.ActivationFunctionType.Exp

    with tile.TileContext(nc) as tc:
        with (
            tc.tile_pool(name="const", bufs=1) as cpool,
            tc.tile_pool(name="stack", bufs=2) as spool,
            tc.tile_pool(name="p1t", bufs=4) as p1pool,
            tc.tile_pool(name="p2t", bufs=6) as p2pool,
            tc.tile_pool(name="epil", bufs=3) as epool,
            tc.tile_pool(name="ps_s", bufs=2, space="PSUM") as ps_s,
            tc.tile_pool(name="ps_o", bufs=2, space="PSUM") as ps_o,
        ):
            rhv_sb = cpool.tile([128, 95], BF16, tag="rhv")
            rwv_sb = cpool.tile([128, 95], BF16, tag="rwv")
            ones1 = cpool.tile([1, 64], BF16, tag="ones1")
            nc.sync.dma_start(rhv_sb[64:128, :], rhv[:, :])
            nc.sync.dma_start(rwv_sb[64:128, :], rwv[:, :])
            nc.gpsimd.memset(ones1[:], 1.0)

            # per-head state dicts
            hs = [dict() for _ in range(HPC)]

            def ph1_load(hh):
                dmae = nc.sync if hh == 0 else nc.gpsimd
                c0, c1 = hh * 64, (hh + 1) * 64
                s = hs[hh]
                s["c0"], s["c1"] = c0, c1
                s["lhsT"] = spool.tile([128, HW], BF16, tag="lhsT", name=f"lhsT{hh}")
                # rq[:, 0, :] = score moving stack (relh rows land in 0:48,
                # zeros 48:64, Q^T 64:128); rq[:, 1, :] = w-major Q^T
                s["rq"] = spool.tile([128, 2, HW], BF16, tag="rq", name=f"rq{hh}")
                s["vt"] = spool.tile([128, KT * 65], BF16, tag="vt", name=f"vt{hh}")
                s["t12"] = spool.tile([95, 2, HW], BF16, tag="t12", name=f"t12{hh}")
                s["relw"] = spool.tile([48, HW], BF16, tag="relw", name=f"relw{hh}")
                s["expw"] = spool.tile([48, HW], BF16, tag="expw", name=f"expw{hh}")
                s["pats"] = spool.tile([128, 3, HW], BF16, tag="pats", name=f"pats{hh}")
                for (l0, ln) in ((0, 768), (768, 768), (1536, 768)):
                    dmae.dma_start(
                        s["rq"][:, :, l0 : l0 + ln],
                        rq_p[:, 2 * hh * HW : 2 * (hh + 1) * HW].rearrange(
                            "p (a b) -> p a b", b=HW
                        )[:, :, l0 : l0 + ln],
                    )
                dmae.dma_start(
                    s["lhsT"][:, :], lhs_p[:, hh * HW : (hh + 1) * HW]
                )
                dmae.dma_start(s["vt"][:, :], v_til[:, hh * KT * 65 : (hh + 1) * KT * 65])

            # gather piece ranges: piece c covers blocks [lo, hi) whose cols
            # fit inside table-store chunks 0..c (512 cols each)
            PIECES = [(0, 10), (10, 21), (21, 32), (32, 42), (42, 48)]

            def ph1_tabs(hh, lo, hi):
                # per chunk: T2w (w-major rel_w table) and T1h (rel_h table)
                # matmuls -> one fused cast -> chunked DRAM stores (ring A)
                # -> gather pieces (ring B: never stuck behind a store).
                # t12[:, 0, :] = T2w, t12[:, 1, :] = T1h.
                s = hs[hh]
                ringA = nc.sync if hh == 0 else nc.gpsimd
                ringB = nc.gpsimd if hh == 0 else nc.sync
                for ci in range(lo, hi):
                    (q0, qn) = QCHUNKS[ci]
                    tp = ps_s.tile([128, 3, 512], F32, tag="s", name=f"tp{hh}_{ci}")
                    nc.tensor.matmul(
                        tp[0:95, 0, 0:qn], rwv_sb[64:128, :],
                        s["rq"][64:128, 1, q0 : q0 + qn], start=True, stop=True,
                    )
                    nc.tensor.matmul(
                        tp[0:95, 1, 0:qn], rhv_sb[64:128, :],
                        s["rq"][64:128, 0, q0 : q0 + qn], start=True, stop=True,
                    )
                    nc.vector.tensor_copy(
                        s["t12"][:, :, q0 : q0 + qn], tp[0:95, 0:2, 0:qn]
                    )
                    for tbl, dram in ((0, t2d[hh]), (1, t1d[hh])):
                        ringA.dma_start(
                            dram[0:95, q0 : q0 + qn], s["t12"][0:95, tbl, q0 : q0 + qn]
                        )
                    # gather pieces (diagonal re-load)
                    (ba, bb) = PIECES[ci]
                    nbl = bb - ba
                    # rel_h: rq[j, 0, (h,w)] = T1h[47-h+j, 48h+w], h in [ba, bb)
                    dsth = s["rq"][0:48, 0, 48 * ba : 48 * bb].rearrange(
                        "p (h w) -> p h w", w=48
                    )
                    srch = t1d[hh][47 - ba : 95 - ba, 48 * ba : 48 * bb].rearrange(
                        "j (h w) -> j h w", w=48
                    )
                    srch.ap[1] = [-2256, nbl]
                    ringB.dma_start(dsth, srch)
                    # rel_w (w-major): relw[j, (w,h)] = T2w[47-w+j, 48w+h]
                    dstw = s["relw"][0:48, 48 * ba : 48 * bb].rearrange(
                        "p (w h) -> p w h", h=48
                    )
                    srcw = t2d[hh][47 - ba : 95 - ba, 48 * ba : 48 * bb].rearrange(
                        "j (w h) -> j w h", h=48
                    )
                    srcw.ap[1] = [-2256, nbl]
                    ringB.dma_start(dstw, srcw)

            def ph1_relw_tail(hh, half):
                # exp with an un-permuting (w-major -> q-major) input AP
                # half=None: full width in one pass (fewer DMAs; for the
                # latency-hidden head)
                s = hs[hh]
                if half is None:
                    h0, co, cn = 0, 0, HW
                    nhb = 48
                else:
                    h0 = half * 24  # h-blocks 0:24/24:48 -> cols 0:1152/1152:2304
                    co, cn = h0 * 48, 1152
                    nhb = 24
                nc.scalar.activation(
                    s["expw"][:, co : co + cn].rearrange("p (h w) -> p h w", w=48),
                    s["relw"][:, :].rearrange("p (w h) -> p h w", w=48)[:, h0 : h0 + nhb, :],
                    Exp,
                )
                ndma = 0
                for j, off in enumerate((0, 32, 16)):
                    p = 0
                    while p < 128:
                        r0 = (p + off) % 48
                        n = min(48 - r0, 128 - p)
                        eng = (nc.sync, nc.gpsimd)[(hh + ndma) % 2]
                        eng.dma_start(
                            s["pats"][p : p + n, j, co : co + cn],
                            s["expw"][r0 : r0 + n, co : co + cn],
                        )
                        p += n
                        ndma += 1

            # ---- main loop machinery (per head) ----
            def make_main(hh):
                s = hs[hh]
                st = {"pend": [], "epiB": [], "g": 0}

                def epilogue_a(ci, q0, qn, o_ps):
                    den = epool.tile([1, 512], F32, tag="den", name=f"den{hh}_{ci}")
                    nc.vector.tensor_copy(den[0:1, 0:qn], o_ps[64:65, 0:qn])
                    dmae = nc.sync if ci % 2 == 0 else nc.gpsimd
                    dent = epool.tile([128, 4], F32, tag="dent", name=f"dent{hh}_{ci}")
                    nr = qn // 128
                    dmae.dma_start(dent[0:128, 0:nr], den[0:1, 0:qn])
                    rect = epool.tile([128, 4], F32, tag="rect", name=f"rect{hh}_{ci}")
                    nc.vector.reciprocal(rect[0:128, 0:nr], dent[0:128, 0:nr])
                    recb = epool.tile([128, 4], BF16, tag="recb", name=f"recb{hh}_{ci}")
                    nc.vector.tensor_copy(recb[0:128, 0:nr], rect[0:128, 0:nr])
                    recr = epool.tile([1, 512], BF16, tag="recr", name=f"recr{hh}_{ci}")
                    dmae.dma_start(recr[0:1, 0:qn], recb[0:128, 0:nr])
                    return recr

                def epilogue_b(ci, q0, qn, o_ps, recr):
                    rb_ps = ps_s.tile([128, 3, 512], F32, tag="s", name=f"rb{hh}_{ci}")
                    nc.tensor.matmul(
                        rb_ps[0:64, 0, 0:qn], ones1[:], recr[0:1, 0:qn],
                        start=True, stop=True,
                    )
                    rb_sb = epool.tile([64, 512], F32, tag="rb_sb", name=f"rbs{hh}_{ci}")
                    nc.vector.tensor_copy(rb_sb[:, 0:qn], rb_ps[0:64, 0, 0:qn])
                    ot = epool.tile([64, 512], F32, tag="ot", name=f"ot{hh}_{ci}")
                    nc.vector.tensor_mul(
                        ot[:, 0:qn], o_ps[0:64, 0:qn], rb_sb[:, 0:qn]
                    )
                    nc.sync.dma_start(out_t[s["c0"] : s["c1"], q0 : q0 + qn], ot[:, 0:qn])

                def flush(last_mm, keep):
                    while len(st["pend"]) > keep:
                        (ci, q0, qn, o_ps, g, p2) = st["pend"].pop(0)
                        for j in range(3):
                            kt = 3 * g + j
                            pv = nc.tensor.matmul(
                                o_ps[0:65, 0:qn],
                                s["vt"][:, kt * 65 : (kt + 1) * 65],
                                p2[:, j, 0:qn],
                                start=(kt == 0), stop=(kt == KT - 1),
                            )
                            if last_mm is not None:
                                add_dep_helper(pv.ins, last_mm.ins, sync=False,
                                               reason="pv after score mms")
                        if g == NG - 1:
                            recr = epilogue_a(ci, q0, qn, o_ps)
                            st["epiB"].append((ci, q0, qn, o_ps, recr, st["g"] + RB_LAG))
                    while st["epiB"] and st["epiB"][0][5] <= st["g"]:
                        (ci, q0, qn, o_ps, recr, _) = st["epiB"].pop(0)
                        epilogue_b(ci, q0, qn, o_ps, recr)

                def chunk(ci):
                    (q0, qn) = QCHUNKS[ci]
                    o_ps = ps_o.tile([65, 512], F32, tag="o", name=f"o{hh}_{ci}")
                    for g in range(NG):
                        s_ps = ps_s.tile([128, 3, 512], F32, tag="s",
                                         name=f"s{hh}_{ci}_{g}")
                        last_mm = None
                        for j in range(3):
                            kt = 3 * g + j
                            last_mm = nc.tensor.matmul(
                                s_ps[:, j, 0:qn],
                                s["lhsT"][:, kt * 128 : (kt + 1) * 128],
                                s["rq"][:, 0, q0 : q0 + qn],
                                start=True, stop=True,
                            )
                        st["g"] += 1
                        flush(last_mm, PV_LAG - 1)
                        p1 = p1pool.tile([128, 3, 512], BF16, tag="p1")
                        nc.scalar.activation(p1[:, :, 0:qn], s_ps[:, :, 0:qn], Exp)
                        p2 = p2pool.tile([128, 3, 512], BF16, tag="p2")
                        nc.vector.tensor_mul(
                            p2[:, :, 0:qn], p1[:, :, 0:qn],
                            s["pats"][:, :, q0 : q0 + qn],
                        )
                        st["pend"].append((ci, q0, qn, o_ps, g, p2))

                def finish_pvs():
                    st["g"] += PV_LAG
                    flush(None, 0)

                def drain(n=1):
                    for _ in range(n):
                        if st["epiB"]:
                            (ci, q0, qn, o_ps, recr, _) = st["epiB"].pop(0)
                            epilogue_b(ci, q0, qn, o_ps, recr)

                return chunk, finish_pvs, drain

            # ---- schedule ----
            ph1_load(0)
            ph1_load(1)
            ph1_tabs(0, 0, 5)
            ph1_relw_tail(0, 0)
            ph1_relw_tail(0, 1)
            chunk0, finish_pvs0, drain0 = make_main(0)
            chunk1, finish_pvs1, drain1 = make_main(1)
            chunk0(0)
            ph1_tabs(1, 0, 3)
            chunk0(1)
            ph1_tabs(1, 3, 5)
            chunk0(2)
            ph1_relw_tail(1, 0)
            ph1_relw_tail(1, 1)
            chunk0(3)
            chunk0(4)
            finish_pvs0()
            chunk1(0)
            drain0(2)
            chunk1(1)
            drain0(2)
            chunk1(2)
            drain0(2)
            chunk1(3)
            chunk1(4)
            finish_pvs1()
            drain1(5)

    nc.compile()
    return nc


def _get_nc():
    global _NC
    if _NC is None:
        _NC = _build_nc()
    return _NC


def _host_prep(q, k, v, rel_pos_h, rel_pos_w):
    q2 = np.asarray(q, np.float32).reshape(HW, NH * DH)
    k2 = np.asarray(k, np.float32).reshape(HW, NH * DH)
    v2 = np.asarray(v, np.float32).reshape(HW, NH * DH)
    rph = np.asarray(rel_pos_h, np.float32)
    rpw = np.asarray(rel_pos_w, np.float32)

    ar = np.arange(48)
    # reversed rel tables, x8 cancels the 0.125 q scale
    rhv = np.ascontiguousarray((8.0 * rph[::-1]).T).astype(BF)   # (64, 95)
    rwv = np.ascontiguousarray((8.0 * rpw[::-1]).T).astype(BF)
    kk = np.arange(HW)
    eh = np.zeros((64, HW), np.float32)
    eh[:48] = kk[None, :] // 48 == ar[:, None]
    eh = eh.astype(BF)

    onecol = np.ones((HW, 1), np.float32)
    in_maps = []
    for c in range(N_CORES):
        sl = slice(c * 128, (c + 1) * 128)
        qs = (q2[:, sl].T * 0.125).astype(BF)                    # (128, HW)
        qw = np.ascontiguousarray(
            qs.reshape(128, 48, 48).transpose(0, 2, 1)
        ).reshape(128, HW)                                       # w-major cols
        ks = k2[:, sl].T.astype(BF)
        lhs_p = np.zeros((128, HPC, HW), BF)
        rq_p = np.zeros((128, HPC, 2, HW), BF)
        vparts = []
        for hh in range(HPC):
            r0, r1 = hh * 64, (hh + 1) * 64
            lhs_p[0:64, hh, :] = eh
            lhs_p[64:128, hh, :] = ks[r0:r1]
            rq_p[64:128, hh, 0, :] = qs[r0:r1]
            rq_p[64:128, hh, 1, :] = qw[r0:r1]
            vh = v2[:, c * 128 + hh * 64 : c * 128 + (hh + 1) * 64]
            va = np.concatenate([vh, onecol], axis=1)            # (HW, 65)
            vparts.append(va.reshape(KT, 128, 65).transpose(1, 0, 2).reshape(128, KT * 65))
        v_til = np.concatenate(vparts, axis=1).astype(BF)        # (128, 2*18*65)
        in_maps.append(
            dict(
                lhs_p=lhs_p.reshape(128, HPC * HW),
                rq_p=rq_p.reshape(128, HPC * 2 * HW),
                v_til=v_til, rhv=rhv, rwv=rwv,
            )
        )
    return in_maps


def _assemble(results):
    cols = [np.asarray(r["out_t"], np.float32).T for r in results]  # (HW, 128) each
    return np.concatenate(cols, axis=1).reshape(1, H, W, NH * DH)


def kernel(q, k, v, rel_pos_h, rel_pos_w):
    nc = _get_nc()
    in_maps = _host_prep(q, k, v, rel_pos_h, rel_pos_w)
    res = run_bass_kernel_spmd(nc, in_maps, core_ids=list(range(N_CORES)))
    return _assemble(res.results)
